# revision 4
# baseline (speedup 1.0000x reference)
"""Depth-weighted 3x3 conv (DepthConv) Trainium2 Bass kernel (V3, fp16/fp8).

Math (per batch element):
  sim[k, p] = exp(-|depth[p + off_k] - depth[p]|)   (9 taps, off = dh*W + dw)
  out[o, p] = sum_{c,k} W[o,c,k] * sim[k,p] * x[c, p + off_k] + bias[o]

Sharding: data-parallel over batch, one batch element per NeuronCore (8).

Per-core layout ("half-image stacking"):
  SBUF partitions = 64 channels x {top half-image, bottom half-image}.
  Free dim = flat padded image: 84 rows x 162 cols (guard row + 82 buffer
  rows + guard row; each row = [pad, 160, pad]).  Output pixel (local row
  j, col w) has center flat index q=(j+1)*162+(w+1) in both halves; tap
  (dh, dw) reads q + dh*162 + dw.

Tap symmetry: sim_{-m}[q] = sim_{+m}[q - off_m], so only 4 similarity maps
exist (center tap's sim == 1).  Per map m:
  tap +m uses xm_m[q]   = x[q+off]*map_m[q]   (x via parity-aligned copy)
  tap -m uses prod_m[q] = x[q]*map_m[q], read by the conv at q - off_m
  (matmul rhs reads have no alignment constraint).

V3 changes vs V2 (V2 was DMA-engine-time bound: ~1.7ms of SDMA engine
time at ~15 GB/s/descriptor):
  - Host pre-casts x/depth/weights to fp16 and pre-pads the image layout:
    the x load halves to 6.97 MB, needs no SWDGE cast, no staging scatter,
    and no guard memsets.
  - The similarity maps are stored in DRAM as fp8e4m3 of (sim - 0.75)
    (numpy-validated: end-to-end rel err 0.93e-2, tol 2e-2).  The
    broadcast DMA casts fp8->fp16 on the fly (read bytes halved), and the
    0.75 shift is re-added for free by computing the products with
    scalar_tensor_tensor((sim' + 0.75) * x).
  - s8d rows are replicated x4 in DRAM (one DRAM->DRAM DMA) so the 64
    partition-replicated broadcast reads spread over 4 copies: V2 showed
    replicated reads at 14.3 GB/s vs 22.5 for unique-address reads (HBM
    address conflicts).

Conv: per 810-px chunk, 9 taps x 2 halves fp16 K=64 matmuls accumulate
into ONE [128, 810] PSUM tile; each (half, bank) range is an independent
accumulation group (has_written clear is per partition -- HW-verified), so
one full-width ACT pass evacuates both halves (+bias, ->fp16).
"""

import functools
import os
import sys

import numpy as np

for _p in ("/opt/trn_rl_repo",):
    if os.path.isdir(_p) and _p not in sys.path:
        sys.path.insert(0, _p)

import concourse.bass as bass
import concourse.mybir as mybir
import concourse.tile as tile
from concourse import bacc
from concourse.bass_utils import run_bass_kernel_spmd

# ---------------------------------------------------------------- constants
B, C, H, W = 8, 64, 160, 160
O = 64
KK = 9
WB = W + 2                 # 162 padded row width
HB = H // 2 + 2            # 82 buffer rows per half
GUARD = WB                 # one padded row of guard cells each side
FLAT = HB * WB             # 13284
FLATG = FLAT + 2 * GUARD   # 13608 (84 rows)
NROWG = FLATG // WB        # 84
Q0 = WB
NCORES = 8

NSEG = 4
SEGROWS = 80 // NSEG       # out-rows per segment (20)
SEGQ = SEGROWS * WB        # 3240
HALO = 164                 # even, >= max |off| (163); low side only
WIN = SEGQ + HALO          # 3404
NCH = 4
CHW = SEGQ // NCH          # 810
SUBS = (512, 298)          # matmul N splits at the fp32 PSUM bank boundary

CW = 851                   # compact-sim width per partition (16*851=13616)
DS = 13824                 # padded row stride of dpad/s8d rows in DRAM
REP = 4                    # DRAM replication of each sim row
SHIFT = 0.75               # sim stored as fp8(sim - SHIFT)

XCH = 4                    # x load column chunks
XCW = FLATG // XCH         # 3402

MAPS = [(0, 1, 1), (1, 0, WB), (1, 1, WB + 1), (1, -1, WB - 1)]

F32 = mybir.dt.float32
F16 = mybir.dt.float16
F8 = mybir.dt.float8e4


def _tapidx(dh, dw):
    return (dh + 1) * 3 + (dw + 1)


def _build_program():
    nc = bacc.Bacc(None)
    x_d = nc.declare_dram_parameter("xpad", [C, 2, FLATG], F16, isOutput=False)
    d_d = nc.declare_dram_parameter("dpad", [2, DS], F16, isOutput=False)
    wt_d = nc.declare_dram_parameter("wt", [C, KK, O], F16, isOutput=False)
    b_d = nc.declare_dram_parameter("bias2", [2 * O], F32, isOutput=False)
    out_d = nc.declare_dram_parameter("out", [O, H, W], F16, isOutput=True)

    Exp = mybir.ActivationFunctionType.Exp
    Ident = mybir.ActivationFunctionType.Identity
    Mult = mybir.AluOpType.mult
    Min = mybir.AluOpType.min
    Add = mybir.AluOpType.add

    with tile.TileContext(nc) as tc:
        with (
            tc.tile_pool(name="dramp", bufs=1, space="DRAM") as dramp,
            tc.tile_pool(name="singles", bufs=1) as singles,
            tc.tile_pool(name="simp", bufs=6) as simp,
            tc.tile_pool(name="prodp", bufs=6) as prodp,
            tc.tile_pool(name="xmp", bufs=5) as xmp,
            tc.tile_pool(name="stgp", bufs=2) as stgp,
            tc.tile_pool(name="cpsum", bufs=4, space="PSUM") as cpsum,
        ):
            x2e = singles.tile([128, FLATG], F16)
            x2o = singles.tile([128, FLATG], F16)
            wt = singles.tile([128, KK, O], F16)
            b2 = singles.tile([128, 1], F32)
            ts8 = singles.tile([128, CW], F16)
            d08 = singles.tile([128, CW], F16)

            # ---------- similarity chain (emitted first: no x dependency)
            # compact shifted depth rows: partition p = m*32 + h*16 + c16
            # holds dpad[h, c16*851 + off_m : +851]; d08 the unshifted rows.
            d_f = d_d[:]
            for m, (dh, dw, off) in enumerate(MAPS):
                nc.scalar.dma_start(
                    out=ts8[m * 32 : (m + 1) * 32, :],
                    in_=bass.AP(
                        tensor=d_f.tensor,
                        offset=d_f.offset + off,
                        ap=[[DS, 2], [CW, 16], [1, CW]],
                    ),
                )
                nc.sync.dma_start(
                    out=d08[m * 32 : (m + 1) * 32, :],
                    in_=bass.AP(
                        tensor=d_f.tensor,
                        offset=d_f.offset,
                        ap=[[DS, 2], [CW, 16], [1, CW]],
                    ),
                )
            nc.vector.tensor_sub(ts8[:], ts8[:], d08[:])
            # -(|t|) = min(-t, t), fused on DVE; exp on ACT; then - SHIFT
            nc.vector.scalar_tensor_tensor(
                ts8[:], ts8[:], -1.0, ts8[:], op0=Mult, op1=Min
            )
            nc.scalar.activation(out=ts8[:], in_=ts8[:], func=Exp, scale=1.0)
            nc.vector.tensor_scalar_sub(ts8[:], ts8[:], SHIFT)
            # fp8 cast into DRAM copy 0, then one DRAM->DRAM DMA fans out
            # REP-1 more copies; layout s8d[m][h][rep][DS].
            s8d = dramp.tile([8, REP, DS], F8)
            s8d_f = s8d[:]
            for m in range(4):
                nc.gpsimd.dma_start(
                    out=bass.AP(
                        tensor=s8d_f.tensor,
                        offset=s8d_f.offset + m * 2 * REP * DS,
                        ap=[[REP * DS, 2], [CW, 16], [1, CW]],
                    ),
                    in_=ts8[m * 32 : (m + 1) * 32, :],
                )
            nc.gpsimd.dma_start(
                out=bass.AP(
                    tensor=s8d_f.tensor,
                    offset=s8d_f.offset + DS,
                    ap=[[REP * DS, 8], [DS, REP - 1], [1, DS]],
                ),
                in_=bass.AP(
                    tensor=s8d_f.tensor,
                    offset=s8d_f.offset,
                    ap=[[REP * DS, 8], [0, REP - 1], [1, DS]],
                ),
            )

            # ---------------- x loads (fp16, host pre-padded; no casts)
            for xc in range(XCH):
                a = xc * XCW
                nc.sync.dma_start(
                    out=x2e[0:64, a : a + XCW], in_=x_d[:, 0, a : a + XCW]
                )
                nc.scalar.dma_start(
                    out=x2e[64:128, a : a + XCW], in_=x_d[:, 1, a : a + XCW]
                )
            # odd-parity copy: x2o[:, j] = x2e[:, j+1], in 4 chunks
            ch4 = (FLATG - 2) // 4 + 1
            for c4 in range(4):
                a4 = c4 * ch4
                b4 = min(FLATG - 2, a4 + ch4)
                nc.sync.dma_start(
                    out=x2o[:, a4:b4], in_=x2e[:, a4 + 1 : b4 + 1]
                )

            nc.sync.dma_start(out=wt[0:64], in_=wt_d[:])
            nc.scalar.dma_start(out=wt[64:128], in_=wt_d[:])
            nc.sync.dma_start(
                out=b2[:], in_=b_d.rearrange("(p one) -> p one", one=1)
            )

            # ---------------- main loop
            for s in range(NSEG):
                qs = Q0 + s * SEGQ
                winbase = GUARD + qs - HALO       # even
                sims = []
                for m, (dh, dw, off) in enumerate(MAPS):
                    sim_m = simp.tile([128, WIN], F16, tag="sim")
                    sims.append(sim_m)
                    for h in range(2):
                        src = bass.AP(
                            tensor=s8d_f.tensor,
                            offset=s8d_f.offset
                            + (m * 2 + h) * REP * DS
                            + winbase,
                            ap=[[DS, REP], [0, 64 // REP], [1, WIN]],
                        )
                        nc.gpsimd.dma_start(
                            out=sim_m[64 * h : 64 * h + 64, :], in_=src
                        )

                prods = []
                xms = []
                for m, (dh, dw, off) in enumerate(MAPS):
                    pr = prodp.tile([128, WIN], F16, tag="prod")
                    prods.append(pr)
                    nc.vector.scalar_tensor_tensor(
                        pr[:],
                        sims[m][:],
                        SHIFT,
                        x2e[:, winbase : winbase + WIN],
                        op0=Add,
                        op1=Mult,
                    )
                    xm = xmp.tile([128, SEGQ], F16, tag="xm")
                    xms.append(xm)
                    if off % 2:
                        xsrc = x2o[
                            :, GUARD + qs + off - 1 : GUARD + qs + off - 1 + SEGQ
                        ]
                    else:
                        xsrc = x2e[:, GUARD + qs + off : GUARD + qs + off + SEGQ]
                    nc.vector.scalar_tensor_tensor(
                        xm[:],
                        sims[m][:, HALO : HALO + SEGQ],
                        SHIFT,
                        xsrc,
                        op0=Add,
                        op1=Mult,
                    )

                stg = stgp.tile([128, SEGROWS * W], F16, tag="stg")
                for j in range(NCH):
                    q = qs + j * CHW
                    so = HALO + j * CHW            # within sims/prods tiles
                    psum = cpsum.tile([128, 1024], F32, tag="cps")
                    o2 = 0
                    for si_, nn2 in enumerate(SUBS):
                        taps = [(_tapidx(0, 0), x2e, GUARD + q + o2)]
                        for m, (dh, dw, off) in enumerate(MAPS):
                            taps.append(
                                (_tapidx(-dh, -dw), prods[m], so - off + o2)
                            )
                        for m, (dh, dw, off) in enumerate(MAPS):
                            taps.append((_tapidx(dh, dw), xms[m], j * CHW + o2))
                        for ti, (widx, rsrc, roff) in enumerate(taps):
                            for half in range(2):
                                pl, ph = 64 * half, 64 * half + 64
                                nc.tensor.matmul(
                                    psum[pl:ph, o2 : o2 + nn2],
                                    wt[pl:ph, widx, :],
                                    rsrc[pl:ph, roff : roff + nn2],
                                    start=(ti == 0),
                                    stop=(ti == len(taps) - 1),
                                    skip_group_check=True,
                                )
                        o2 += nn2
                    # CHW = 810 = 5 padded rows; strip the pad columns in
                    # the evacuation (strided psum read, contiguous out)
                    nc.scalar.activation(
                        out=stg[
                            :, j * 5 * W : (j + 1) * 5 * W
                        ].rearrange("p (r w) -> p r w", r=5, w=W),
                        in_=bass.AP(
                            tensor=psum[:].tensor,
                            offset=psum[:].offset + 1,
                            ap=[list(psum[:].ap[0]), [WB, 5], [1, W]],
                        ),
                        func=Ident,
                        bias=b2[:],
                        scale=1.0,
                    )

                r0 = SEGROWS * s
                nc.sync.dma_start(
                    out=out_d[:, r0 : r0 + SEGROWS, :].rearrange(
                        "c r w -> c (r w)"
                    ),
                    in_=stg[0:64, :],
                )
                nc.scalar.dma_start(
                    out=out_d[:, 80 + r0 : 80 + r0 + SEGROWS, :].rearrange(
                        "c r w -> c (r w)"
                    ),
                    in_=stg[64:128, :],
                )

    return nc


@functools.lru_cache(maxsize=1)
def _get_program():
    return _build_program()


def make_in_maps(x, depth, weights, bias):
    wt = np.ascontiguousarray(
        weights.reshape(O, C, KK).transpose(1, 2, 0)
    ).astype(np.float16)
    b2 = np.concatenate([bias, bias]).astype(np.float32)
    base = {"wt": wt, "bias2": b2}
    maps = []
    for i in range(x.shape[0]):
        xi = np.asarray(x[i], np.float32)
        xpad = np.zeros((C, 2, NROWG, WB), np.float16)
        xpad[:, 0, 2:83, 1:161] = xi[:, 0:81, :]
        xpad[:, 1, 1:82, 1:161] = xi[:, 79:160, :]
        di = np.asarray(depth[i, 0], np.float32)
        dpad = np.zeros((2, DS), np.float16)
        dv = dpad[:, 0:FLATG].reshape(2, NROWG, WB)
        dv[0, 2:83, 1:161] = di[0:81, :]
        dv[1, 1:82, 1:161] = di[79:160, :]
        maps.append(
            {
                "xpad": xpad.reshape(C, 2, FLATG),
                "dpad": dpad,
                **base,
            }
        )
    return maps


def kernel(x, depth, weights, bias):
    nc = _get_program()
    if not nc.is_finalized():
        nc.finalize()
    in_maps = make_in_maps(x, depth, weights, bias)
    res = run_bass_kernel_spmd(nc, in_maps, list(range(NCORES)))
    out = np.stack([np.asarray(res.results[i]["out"]) for i in range(NCORES)])
    return out.astype(np.float32)


# revision 8
# speedup vs baseline: 1.1139x; 1.1139x over previous
"""Depth-weighted 3x3 conv (DepthConv) Trainium2 Bass kernel (V3, fp16/fp8).

Math (per batch element):
  sim[k, p] = exp(-|depth[p + off_k] - depth[p]|)   (9 taps, off = dh*W + dw)
  out[o, p] = sum_{c,k} W[o,c,k] * sim[k,p] * x[c, p + off_k] + bias[o]

Sharding: data-parallel over batch, one batch element per NeuronCore (8).

Per-core layout ("half-image stacking"):
  SBUF partitions = 64 channels x {top half-image, bottom half-image}.
  Free dim = flat padded image: 84 rows x 162 cols (guard row + 82 buffer
  rows + guard row; each row = [pad, 160, pad]).  Output pixel (local row
  j, col w) has center flat index q=(j+1)*162+(w+1) in both halves; tap
  (dh, dw) reads q + dh*162 + dw.

Tap symmetry: sim_{-m}[q] = sim_{+m}[q - off_m], so only 4 similarity maps
exist (center tap's sim == 1).  Per map m:
  tap +m uses xm_m[q]   = x[q+off]*map_m[q]   (x via parity-aligned copy)
  tap -m uses prod_m[q] = x[q]*map_m[q], read by the conv at q - off_m
  (matmul rhs reads have no alignment constraint).

V3 changes vs V2 (V2 was DMA-engine-time bound: ~1.7ms of SDMA engine
time at ~15 GB/s/descriptor):
  - Host pre-casts x/depth/weights to fp16 and pre-pads the image layout:
    the x load halves to 6.97 MB, needs no SWDGE cast, no staging scatter,
    and no guard memsets.
  - The similarity maps are stored in DRAM as fp8e4m3 directly
    (numpy-validated: end-to-end rel err 1.73e-2, tol 2e-2; HW DMA cast
    measured RTNE-consistent).  The broadcast DMA casts fp8->fp16 on the
    fly (read bytes halved); products stay plain 2x-mode tensor_mul
    (scalar_tensor_tensor was measured 1x — a 2x DVE regression).
  - s8d rows are replicated x8 in DRAM (one DRAM->DRAM DMA) so the 64
    partition-replicated broadcast reads spread over 8 copies: V2 showed
    replicated reads at 14.3 GB/s vs 22.5 for unique-address reads (HBM
    address conflicts).

Conv: per 810-px chunk, 9 taps x 2 halves fp16 K=64 matmuls accumulate
into ONE [128, 810] PSUM tile; each (half, bank) range is an independent
accumulation group (has_written clear is per partition -- HW-verified), so
one full-width ACT pass evacuates both halves (+bias, ->fp16).
"""

import functools
import os
import sys

import numpy as np

for _p in ("/opt/trn_rl_repo",):
    if os.path.isdir(_p) and _p not in sys.path:
        sys.path.insert(0, _p)

import concourse.bass as bass
import concourse.mybir as mybir
import concourse.tile as tile
from concourse import bacc
from concourse.bass_utils import run_bass_kernel_spmd

# ---------------------------------------------------------------- constants
B, C, H, W = 8, 64, 160, 160
O = 64
KK = 9
WB = W + 2                 # 162 padded row width
HB = H // 2 + 2            # 82 buffer rows per half
GUARD = WB                 # one padded row of guard cells each side
FLAT = HB * WB             # 13284
FLATG = FLAT + 2 * GUARD   # 13608 (84 rows)
NROWG = FLATG // WB        # 84
Q0 = WB
NCORES = 8

NSEG = 4
SEGROWS = 80 // NSEG       # out-rows per segment (20)
SEGQ = SEGROWS * WB        # 3240
HALO = 164                 # even, >= max |off| (163); low side only
WIN = SEGQ + HALO          # 3404
NCH = 4
CHW = SEGQ // NCH          # 810
SUBS = (512, 298)          # matmul N splits at the fp32 PSUM bank boundary

CW = 851                   # compact-sim width per partition (16*851=13616)
DS = 13824                 # padded row stride of dpad/s8d rows in DRAM
REP = 8                    # DRAM replication of each sim row

XCH = 4                    # x load column chunks
XCW = FLATG // XCH         # 3402

MAPS = [(0, 1, 1), (1, 0, WB), (1, 1, WB + 1), (1, -1, WB - 1)]

F32 = mybir.dt.float32
F16 = mybir.dt.float16
F8 = mybir.dt.float8e4


def _tapidx(dh, dw):
    return (dh + 1) * 3 + (dw + 1)


def _build_program():
    nc = bacc.Bacc(None)
    x_d = nc.declare_dram_parameter("xpad", [C, 2, FLATG], F16, isOutput=False)
    d_d = nc.declare_dram_parameter("dpad", [2, DS], F16, isOutput=False)
    wt_d = nc.declare_dram_parameter("wt", [C, KK, O], F16, isOutput=False)
    b_d = nc.declare_dram_parameter("bias2", [2 * O], F32, isOutput=False)
    out_d = nc.declare_dram_parameter("out", [O, H, W], F16, isOutput=True)

    Exp = mybir.ActivationFunctionType.Exp
    Ident = mybir.ActivationFunctionType.Identity
    Mult = mybir.AluOpType.mult
    Min = mybir.AluOpType.min
    Add = mybir.AluOpType.add

    with tile.TileContext(nc) as tc:
        with (
            tc.tile_pool(name="dramp", bufs=1, space="DRAM") as dramp,
            tc.tile_pool(name="singles", bufs=1) as singles,
            tc.tile_pool(name="simp", bufs=6) as simp,
            tc.tile_pool(name="prodp", bufs=6) as prodp,
            tc.tile_pool(name="xmp", bufs=5) as xmp,
            tc.tile_pool(name="stgp", bufs=2) as stgp,
            tc.tile_pool(name="cpsum", bufs=4, space="PSUM") as cpsum,
        ):
            x2e = singles.tile([128, FLATG], F16)
            x2o = singles.tile([128, FLATG], F16)
            wt = singles.tile([128, KK, O], F16)
            b2 = singles.tile([128, 1], F32)
            ts8 = singles.tile([128, CW], F16)
            d08 = singles.tile([128, CW], F16)

            # ---------- similarity chain (emitted first: no x dependency)
            # compact shifted depth rows: partition p = m*32 + h*16 + c16
            # holds dpad[h, c16*851 + off_m : +851]; d08 the unshifted rows.
            d_f = d_d[:]
            for m, (dh, dw, off) in enumerate(MAPS):
                nc.scalar.dma_start(
                    out=ts8[m * 32 : (m + 1) * 32, :],
                    in_=bass.AP(
                        tensor=d_f.tensor,
                        offset=d_f.offset + off,
                        ap=[[DS, 2], [CW, 16], [1, CW]],
                    ),
                )
                nc.sync.dma_start(
                    out=d08[m * 32 : (m + 1) * 32, :],
                    in_=bass.AP(
                        tensor=d_f.tensor,
                        offset=d_f.offset,
                        ap=[[DS, 2], [CW, 16], [1, CW]],
                    ),
                )
            nc.vector.tensor_sub(ts8[:], ts8[:], d08[:])
            # -(|t|) = min(-t, t), fused on DVE; exp on ACT
            nc.vector.scalar_tensor_tensor(
                ts8[:], ts8[:], -1.0, ts8[:], op0=Mult, op1=Min
            )
            nc.scalar.activation(out=ts8[:], in_=ts8[:], func=Exp, scale=1.0)
            # fp8 cast into DRAM copy 0, then one DRAM->DRAM DMA fans out
            # REP-1 more copies; layout s8d[m][h][rep][DS].
            s8d = dramp.tile([8, REP, DS], F8)
            s8d_f = s8d[:]
            for m in range(4):
                nc.gpsimd.dma_start(
                    out=bass.AP(
                        tensor=s8d_f.tensor,
                        offset=s8d_f.offset + m * 2 * REP * DS,
                        ap=[[REP * DS, 2], [CW, 16], [1, CW]],
                    ),
                    in_=ts8[m * 32 : (m + 1) * 32, :],
                )
            nc.gpsimd.dma_start(
                out=bass.AP(
                    tensor=s8d_f.tensor,
                    offset=s8d_f.offset + DS,
                    ap=[[REP * DS, 8], [DS, REP - 1], [1, DS]],
                ),
                in_=bass.AP(
                    tensor=s8d_f.tensor,
                    offset=s8d_f.offset,
                    ap=[[REP * DS, 8], [0, REP - 1], [1, DS]],
                ),
            )

            # ---------------- x loads (fp16, host pre-padded; no casts)
            for xc in range(XCH):
                a = xc * XCW
                nc.sync.dma_start(
                    out=x2e[0:64, a : a + XCW], in_=x_d[:, 0, a : a + XCW]
                )
                nc.scalar.dma_start(
                    out=x2e[64:128, a : a + XCW], in_=x_d[:, 1, a : a + XCW]
                )
            # odd-parity copy: x2o[:, j] = x2e[:, j+1], in 4 chunks
            ch4 = (FLATG - 2) // 4 + 1
            for c4 in range(4):
                a4 = c4 * ch4
                b4 = min(FLATG - 2, a4 + ch4)
                nc.sync.dma_start(
                    out=x2o[:, a4:b4], in_=x2e[:, a4 + 1 : b4 + 1]
                )

            nc.sync.dma_start(out=wt[0:64], in_=wt_d[:])
            nc.scalar.dma_start(out=wt[64:128], in_=wt_d[:])
            nc.sync.dma_start(
                out=b2[:], in_=b_d.rearrange("(p one) -> p one", one=1)
            )

            # ---------------- main loop
            for s in range(NSEG):
                qs = Q0 + s * SEGQ
                winbase = GUARD + qs - HALO       # even
                sims = []
                for m, (dh, dw, off) in enumerate(MAPS):
                    sim_m = simp.tile([128, WIN], F16, tag="sim")
                    sims.append(sim_m)
                    for h in range(2):
                        src = bass.AP(
                            tensor=s8d_f.tensor,
                            offset=s8d_f.offset
                            + (m * 2 + h) * REP * DS
                            + winbase,
                            ap=[[DS, REP], [0, 64 // REP], [1, WIN]],
                        )
                        nc.gpsimd.dma_start(
                            out=sim_m[64 * h : 64 * h + 64, :], in_=src
                        )

                prods = []
                xms = []
                for m, (dh, dw, off) in enumerate(MAPS):
                    pr = prodp.tile([128, WIN], F16, tag="prod")
                    prods.append(pr)
                    nc.vector.tensor_mul(
                        pr[:], x2e[:, winbase : winbase + WIN], sims[m][:]
                    )
                    xm = xmp.tile([128, SEGQ], F16, tag="xm")
                    xms.append(xm)
                    if off % 2:
                        xsrc = x2o[
                            :, GUARD + qs + off - 1 : GUARD + qs + off - 1 + SEGQ
                        ]
                    else:
                        xsrc = x2e[:, GUARD + qs + off : GUARD + qs + off + SEGQ]
                    nc.vector.tensor_mul(
                        xm[:], xsrc, sims[m][:, HALO : HALO + SEGQ]
                    )

                stg = stgp.tile([128, SEGROWS * W], F16, tag="stg")
                for j in range(NCH):
                    q = qs + j * CHW
                    so = HALO + j * CHW            # within sims/prods tiles
                    psum = cpsum.tile([128, 1024], F32, tag="cps")
                    o2 = 0
                    for si_, nn2 in enumerate(SUBS):
                        taps = [(_tapidx(0, 0), x2e, GUARD + q + o2)]
                        for m, (dh, dw, off) in enumerate(MAPS):
                            taps.append(
                                (_tapidx(-dh, -dw), prods[m], so - off + o2)
                            )
                        for m, (dh, dw, off) in enumerate(MAPS):
                            taps.append((_tapidx(dh, dw), xms[m], j * CHW + o2))
                        for ti, (widx, rsrc, roff) in enumerate(taps):
                            for half in range(2):
                                pl, ph = 64 * half, 64 * half + 64
                                nc.tensor.matmul(
                                    psum[pl:ph, o2 : o2 + nn2],
                                    wt[pl:ph, widx, :],
                                    rsrc[pl:ph, roff : roff + nn2],
                                    start=(ti == 0),
                                    stop=(ti == len(taps) - 1),
                                    skip_group_check=True,
                                )
                        o2 += nn2
                    # CHW = 810 = 5 padded rows; strip the pad columns in
                    # the evacuation (strided psum read, contiguous out)
                    nc.scalar.activation(
                        out=stg[
                            :, j * 5 * W : (j + 1) * 5 * W
                        ].rearrange("p (r w) -> p r w", r=5, w=W),
                        in_=bass.AP(
                            tensor=psum[:].tensor,
                            offset=psum[:].offset + 1,
                            ap=[list(psum[:].ap[0]), [WB, 5], [1, W]],
                        ),
                        func=Ident,
                        bias=b2[:],
                        scale=1.0,
                    )

                r0 = SEGROWS * s
                nc.sync.dma_start(
                    out=out_d[:, r0 : r0 + SEGROWS, :].rearrange(
                        "c r w -> c (r w)"
                    ),
                    in_=stg[0:64, :],
                )
                nc.scalar.dma_start(
                    out=out_d[:, 80 + r0 : 80 + r0 + SEGROWS, :].rearrange(
                        "c r w -> c (r w)"
                    ),
                    in_=stg[64:128, :],
                )

    return nc


@functools.lru_cache(maxsize=1)
def _get_program():
    return _build_program()


def make_in_maps(x, depth, weights, bias):
    wt = np.ascontiguousarray(
        weights.reshape(O, C, KK).transpose(1, 2, 0)
    ).astype(np.float16)
    b2 = np.concatenate([bias, bias]).astype(np.float32)
    base = {"wt": wt, "bias2": b2}
    maps = []
    for i in range(x.shape[0]):
        xi = np.asarray(x[i], np.float32)
        xpad = np.zeros((C, 2, NROWG, WB), np.float16)
        xpad[:, 0, 2:83, 1:161] = xi[:, 0:81, :]
        xpad[:, 1, 1:82, 1:161] = xi[:, 79:160, :]
        di = np.asarray(depth[i, 0], np.float32)
        dpad = np.zeros((2, DS), np.float16)
        dv = dpad[:, 0:FLATG].reshape(2, NROWG, WB)
        dv[0, 2:83, 1:161] = di[0:81, :]
        dv[1, 1:82, 1:161] = di[79:160, :]
        maps.append(
            {
                "xpad": xpad.reshape(C, 2, FLATG),
                "dpad": dpad,
                **base,
            }
        )
    return maps


def kernel(x, depth, weights, bias):
    nc = _get_program()
    if not nc.is_finalized():
        nc.finalize()
    in_maps = make_in_maps(x, depth, weights, bias)
    res = run_bass_kernel_spmd(nc, in_maps, list(range(NCORES)))
    out = np.stack([np.asarray(res.results[i]["out"]) for i in range(NCORES)])
    return out.astype(np.float32)


# revision 11
# speedup vs baseline: 1.2404x; 1.1136x over previous
"""Depth-weighted 3x3 conv (DepthConv) Trainium2 Bass kernel (V3, fp16/fp8).

Math (per batch element):
  sim[k, p] = exp(-|depth[p + off_k] - depth[p]|)   (9 taps, off = dh*W + dw)
  out[o, p] = sum_{c,k} W[o,c,k] * sim[k,p] * x[c, p + off_k] + bias[o]

Sharding: data-parallel over batch, one batch element per NeuronCore (8).

Per-core layout ("half-image stacking"):
  SBUF partitions = 64 channels x {top half-image, bottom half-image}.
  Free dim = flat padded image: 84 rows x 162 cols (guard row + 82 buffer
  rows + guard row; each row = [pad, 160, pad]).  Output pixel (local row
  j, col w) has center flat index q=(j+1)*162+(w+1) in both halves; tap
  (dh, dw) reads q + dh*162 + dw.

Tap symmetry: sim_{-m}[q] = sim_{+m}[q - off_m], so only 4 similarity maps
exist (center tap's sim == 1).  Per map m:
  tap +m uses xm_m[q]   = x[q+off]*map_m[q]   (x via parity-aligned copy)
  tap -m uses prod_m[q] = x[q]*map_m[q], read by the conv at q - off_m
  (matmul rhs reads have no alignment constraint).

V3 changes vs V2 (V2 was DMA-engine-time bound: ~1.7ms of SDMA engine
time at ~15 GB/s/descriptor):
  - Host pre-casts x/depth/weights to fp16 and pre-pads the image layout:
    the x load halves to 6.97 MB, needs no SWDGE cast, no staging scatter,
    and no guard memsets.
  - The similarity maps are stored in DRAM as fp8e4m3 directly
    (numpy-validated: end-to-end rel err 1.73e-2, tol 2e-2; HW DMA cast
    measured RTNE-consistent).  The broadcast DMA casts fp8->fp16 on the
    fly (read bytes halved); products stay plain 2x-mode tensor_mul
    (scalar_tensor_tensor was measured 1x — a 2x DVE regression).
  - s8d rows are replicated x8 in DRAM (one DRAM->DRAM DMA) so the 64
    partition-replicated broadcast reads spread over 8 copies: V2 showed
    replicated reads at 14.3 GB/s vs 22.5 for unique-address reads (HBM
    address conflicts).

Conv: per 810-px chunk, 9 taps x 2 halves fp16 K=64 matmuls accumulate
into ONE [128, 810] PSUM tile; each (half, bank) range is an independent
accumulation group (has_written clear is per partition -- HW-verified), so
one full-width ACT pass evacuates both halves (+bias, ->fp16).
"""

import functools
import os
import sys

import numpy as np

for _p in ("/opt/trn_rl_repo",):
    if os.path.isdir(_p) and _p not in sys.path:
        sys.path.insert(0, _p)

import concourse.bass as bass
import concourse.mybir as mybir
import concourse.tile as tile
from concourse import bacc
from concourse.bass_utils import run_bass_kernel_spmd

# ---------------------------------------------------------------- constants
B, C, H, W = 8, 64, 160, 160
O = 64
KK = 9
WB = W + 2                 # 162 padded row width
HB = H // 2 + 2            # 82 buffer rows per half
GUARD = WB                 # one padded row of guard cells each side
FLAT = HB * WB             # 13284
FLATG = FLAT + 2 * GUARD   # 13608 (84 rows)
NROWG = FLATG // WB        # 84
Q0 = WB
NCORES = 8

NSEG = 4
SEGROWS = 80 // NSEG       # out-rows per segment (20)
SEGQ = SEGROWS * WB        # 3240
HALO = 164                 # even, >= max |off| (163); low side only
WIN = SEGQ + HALO          # 3404
NCH = 4
CHW = SEGQ // NCH          # 810
SUBS = (512, 298)          # matmul N splits at the fp32 PSUM bank boundary

CW = 851                   # compact-sim width per partition (16*851=13616)
DS = 13824                 # padded row stride of dpad/s8d rows in DRAM
REP = 8                    # DRAM replication of each sim row
SHIFT = 0.75               # sim stored as fp8(sim - SHIFT); restored on DVE

XCH = 4                    # x load column chunks
XCW = FLATG // XCH         # 3402

MAPS = [(0, 1, 1), (1, 0, WB), (1, 1, WB + 1), (1, -1, WB - 1)]

F32 = mybir.dt.float32
F16 = mybir.dt.float16
F8 = mybir.dt.float8e4


def _tapidx(dh, dw):
    return (dh + 1) * 3 + (dw + 1)


def _build_program():
    nc = bacc.Bacc(None)
    x_d = nc.declare_dram_parameter("xpad", [C, 2, FLATG], F16, isOutput=False)
    d_d = nc.declare_dram_parameter("dpad", [2, DS], F16, isOutput=False)
    wt_d = nc.declare_dram_parameter("wt", [C, KK, O], F16, isOutput=False)
    b_d = nc.declare_dram_parameter("bias2", [2 * O], F32, isOutput=False)
    out_d = nc.declare_dram_parameter("out", [O, H, W], F16, isOutput=True)

    Exp = mybir.ActivationFunctionType.Exp
    Ident = mybir.ActivationFunctionType.Identity
    Mult = mybir.AluOpType.mult
    Min = mybir.AluOpType.min
    Add = mybir.AluOpType.add

    with tile.TileContext(nc) as tc:
        with (
            tc.tile_pool(name="dramp", bufs=1, space="DRAM") as dramp,
            tc.tile_pool(name="singles", bufs=1) as singles,
            tc.tile_pool(name="simp", bufs=6) as simp,
            tc.tile_pool(name="prodp", bufs=6) as prodp,
            tc.tile_pool(name="xmp", bufs=5) as xmp,
            tc.tile_pool(name="stgp", bufs=2) as stgp,
            tc.tile_pool(name="cpsum", bufs=4, space="PSUM") as cpsum,
        ):
            x2e = singles.tile([128, FLATG], F16)
            x2o = singles.tile([128, FLATG], F16)
            wt = singles.tile([128, KK, O], F16)
            b2 = singles.tile([128, 1], F32)
            ts8 = singles.tile([128, CW], F16)
            d08 = singles.tile([128, CW], F16)

            # ---------- similarity chain (emitted first: no x dependency)
            # compact shifted depth rows: partition p = m*32 + h*16 + c16
            # holds dpad[h, c16*851 + off_m : +851]; d08 the unshifted rows.
            d_f = d_d[:]
            for m, (dh, dw, off) in enumerate(MAPS):
                nc.scalar.dma_start(
                    out=ts8[m * 32 : (m + 1) * 32, :],
                    in_=bass.AP(
                        tensor=d_f.tensor,
                        offset=d_f.offset + off,
                        ap=[[DS, 2], [CW, 16], [1, CW]],
                    ),
                )
                nc.sync.dma_start(
                    out=d08[m * 32 : (m + 1) * 32, :],
                    in_=bass.AP(
                        tensor=d_f.tensor,
                        offset=d_f.offset,
                        ap=[[DS, 2], [CW, 16], [1, CW]],
                    ),
                )
            nc.vector.tensor_sub(ts8[:], ts8[:], d08[:])
            # -(|t|) = min(-t, t), fused on DVE; exp on ACT
            nc.vector.scalar_tensor_tensor(
                ts8[:], ts8[:], -1.0, ts8[:], op0=Mult, op1=Min
            )
            nc.scalar.activation(out=ts8[:], in_=ts8[:], func=Exp, scale=1.0)
            nc.vector.tensor_scalar_sub(ts8[:], ts8[:], SHIFT)
            # fp8 cast into DRAM copy 0, then one DRAM->DRAM DMA fans out
            # REP-1 more copies; layout s8d[m][h][rep][DS].
            s8d = dramp.tile([8, REP, DS], F8)
            s8d_f = s8d[:]
            for m in range(4):
                nc.gpsimd.dma_start(
                    out=bass.AP(
                        tensor=s8d_f.tensor,
                        offset=s8d_f.offset + m * 2 * REP * DS,
                        ap=[[REP * DS, 2], [CW, 16], [1, CW]],
                    ),
                    in_=ts8[m * 32 : (m + 1) * 32, :],
                )
            nc.gpsimd.dma_start(
                out=bass.AP(
                    tensor=s8d_f.tensor,
                    offset=s8d_f.offset + DS,
                    ap=[[REP * DS, 8], [DS, REP - 1], [1, DS]],
                ),
                in_=bass.AP(
                    tensor=s8d_f.tensor,
                    offset=s8d_f.offset,
                    ap=[[REP * DS, 8], [0, REP - 1], [1, DS]],
                ),
            )

            # ---------------- x loads (fp16, host pre-padded; no casts)
            for xc in range(XCH):
                a = xc * XCW
                nc.sync.dma_start(
                    out=x2e[0:64, a : a + XCW], in_=x_d[:, 0, a : a + XCW]
                )
                nc.scalar.dma_start(
                    out=x2e[64:128, a : a + XCW], in_=x_d[:, 1, a : a + XCW]
                )
            # odd-parity copy: x2o[:, j] = x2e[:, j+1], in 4 chunks
            ch4 = (FLATG - 2) // 4 + 1
            for c4 in range(4):
                a4 = c4 * ch4
                b4 = min(FLATG - 2, a4 + ch4)
                nc.sync.dma_start(
                    out=x2o[:, a4:b4], in_=x2e[:, a4 + 1 : b4 + 1]
                )

            nc.sync.dma_start(out=wt[0:64], in_=wt_d[:])
            nc.scalar.dma_start(out=wt[64:128], in_=wt_d[:])
            nc.sync.dma_start(
                out=b2[:], in_=b_d.rearrange("(p one) -> p one", one=1)
            )

            # ---------------- main loop
            for s in range(NSEG):
                qs = Q0 + s * SEGQ
                winbase = GUARD + qs - HALO       # even
                sims = []
                for m, (dh, dw, off) in enumerate(MAPS):
                    sim_m = simp.tile([128, WIN], F16, tag="sim")
                    sims.append(sim_m)
                    for h in range(2):
                        src = bass.AP(
                            tensor=s8d_f.tensor,
                            offset=s8d_f.offset
                            + (m * 2 + h) * REP * DS
                            + winbase,
                            ap=[[DS, REP], [0, 64 // REP], [1, WIN]],
                        )
                        nc.gpsimd.dma_start(
                            out=sim_m[64 * h : 64 * h + 64, :], in_=src
                        )
                    # restore the fp8 range shift (4x-mode tensor_scalar)
                    nc.vector.tensor_scalar_add(sim_m[:], sim_m[:], SHIFT)

                prods = []
                xms = []
                for m, (dh, dw, off) in enumerate(MAPS):
                    pr = prodp.tile([128, WIN], F16, tag="prod")
                    prods.append(pr)
                    nc.vector.tensor_mul(
                        pr[:], x2e[:, winbase : winbase + WIN], sims[m][:]
                    )
                    xm = xmp.tile([128, SEGQ], F16, tag="xm")
                    xms.append(xm)
                    if off % 2:
                        xsrc = x2o[
                            :, GUARD + qs + off - 1 : GUARD + qs + off - 1 + SEGQ
                        ]
                    else:
                        xsrc = x2e[:, GUARD + qs + off : GUARD + qs + off + SEGQ]
                    nc.vector.tensor_mul(
                        xm[:], xsrc, sims[m][:, HALO : HALO + SEGQ]
                    )

                stg = stgp.tile([128, SEGROWS * W], F16, tag="stg")
                for j in range(NCH):
                    q = qs + j * CHW
                    so = HALO + j * CHW            # within sims/prods tiles
                    psum = cpsum.tile([128, 1024], F32, tag="cps")
                    o2 = 0
                    for si_, nn2 in enumerate(SUBS):
                        taps = [(_tapidx(0, 0), x2e, GUARD + q + o2)]
                        for m, (dh, dw, off) in enumerate(MAPS):
                            taps.append(
                                (_tapidx(-dh, -dw), prods[m], so - off + o2)
                            )
                        for m, (dh, dw, off) in enumerate(MAPS):
                            taps.append((_tapidx(dh, dw), xms[m], j * CHW + o2))
                        for ti, (widx, rsrc, roff) in enumerate(taps):
                            for half in range(2):
                                pl, ph = 64 * half, 64 * half + 64
                                nc.tensor.matmul(
                                    psum[pl:ph, o2 : o2 + nn2],
                                    wt[pl:ph, widx, :],
                                    rsrc[pl:ph, roff : roff + nn2],
                                    start=(ti == 0),
                                    stop=(ti == len(taps) - 1),
                                    skip_group_check=True,
                                )
                        o2 += nn2
                    # CHW = 810 = 5 padded rows; strip the pad columns in
                    # the evacuation (strided psum read, contiguous out)
                    nc.scalar.activation(
                        out=stg[
                            :, j * 5 * W : (j + 1) * 5 * W
                        ].rearrange("p (r w) -> p r w", r=5, w=W),
                        in_=bass.AP(
                            tensor=psum[:].tensor,
                            offset=psum[:].offset + 1,
                            ap=[list(psum[:].ap[0]), [WB, 5], [1, W]],
                        ),
                        func=Ident,
                        bias=b2[:],
                        scale=1.0,
                    )

                r0 = SEGROWS * s
                nc.sync.dma_start(
                    out=out_d[:, r0 : r0 + SEGROWS, :].rearrange(
                        "c r w -> c (r w)"
                    ),
                    in_=stg[0:64, :],
                )
                nc.scalar.dma_start(
                    out=out_d[:, 80 + r0 : 80 + r0 + SEGROWS, :].rearrange(
                        "c r w -> c (r w)"
                    ),
                    in_=stg[64:128, :],
                )

    return nc


@functools.lru_cache(maxsize=1)
def _get_program():
    return _build_program()


def make_in_maps(x, depth, weights, bias):
    wt = np.ascontiguousarray(
        weights.reshape(O, C, KK).transpose(1, 2, 0)
    ).astype(np.float16)
    b2 = np.concatenate([bias, bias]).astype(np.float32)
    base = {"wt": wt, "bias2": b2}
    maps = []
    for i in range(x.shape[0]):
        xi = np.asarray(x[i], np.float32)
        xpad = np.zeros((C, 2, NROWG, WB), np.float16)
        xpad[:, 0, 2:83, 1:161] = xi[:, 0:81, :]
        xpad[:, 1, 1:82, 1:161] = xi[:, 79:160, :]
        di = np.asarray(depth[i, 0], np.float32)
        dpad = np.zeros((2, DS), np.float16)
        dv = dpad[:, 0:FLATG].reshape(2, NROWG, WB)
        dv[0, 2:83, 1:161] = di[0:81, :]
        dv[1, 1:82, 1:161] = di[79:160, :]
        maps.append(
            {
                "xpad": xpad.reshape(C, 2, FLATG),
                "dpad": dpad,
                **base,
            }
        )
    return maps


def kernel(x, depth, weights, bias):
    nc = _get_program()
    if not nc.is_finalized():
        nc.finalize()
    in_maps = make_in_maps(x, depth, weights, bias)
    res = run_bass_kernel_spmd(nc, in_maps, list(range(NCORES)))
    out = np.stack([np.asarray(res.results[i]["out"]) for i in range(NCORES)])
    return out.astype(np.float32)
